# revision 31
# baseline (speedup 1.0000x reference)
"""GCN 2-layer (PyG GCNConv x2) Trainium2 kernel, 8-core SPMD.

Strategy:
  - Shard destination nodes across the 8 cores (12500 each). Weights
    replicated. SWDGE descriptor generation is the machine bottleneck, so
    gathers/scatter-free combines round-robin over 4 SWDGE queues and the
    descriptor count is minimized.
  - Windows are source QUARTERS of every rank so the per-layer AllGather
    runs as 4 pipelined quarter-collectives overlapping phase compute and
    the first gather passes.
  - Layer l: each core computes its shard of the scaled features
    (h'' = (x @ W) * dinv for layer 1, hid'' = leakyrelu(...)*dinv for 2),
    kept resident in SBUF (self-loop term) and quarter-AllGathered into
    per-window tables in HBM.
  - Message gather: MoE dma_gather (int16 idxs) from the window table.
    Per (core, window) pass, destinations are sorted by in-window degree
    and laid out on the 128 SBUF partitions; each block of 128 destinations
    has a uniform round count R (max degree in the block); gathered
    messages land [128 lanes, R rounds, 64] and a strided reduce_sum
    collapses R into per-block partial sums.
  - Partial blocks are bulk-written (HWDGE, no descriptors on the SWDGE
    path) to per-pass-pair "rod" DRAM tensors. The combine step then
    gathers, per destination row, its 4 per-pass partials (one descriptor
    per row-pass instead of a 2-descriptor RMW scatter per row) and sums
    them with the SBUF-resident self term — fused into phase D (layer
    boundary) and phase G (output).
  - Layer 2 reuses the identical edge schedule/index tensors on the hid''
    table; final output = (acc2 * dinv) @ W2 + b2 via PE transpose+matmul.
"""

import numpy as np


# ---------------------------------------------------------------- config

class Cfg:
    """Windows are source-QUARTERS of every rank (quarter q of each shard),
    so AllGather can run as 4 pipelined quarter-collectives.

    Window q's table = concat over ranks of [qsize[q] rows + pad zeros].
    """
    def __init__(self, N=100000, E=1200000, ncores=8, Q=4, tile_cols=32,
                 din=128, dh=64, dout=40):
        assert N % ncores == 0
        self.N, self.E, self.ncores, self.Q = N, E, ncores, Q
        self.shard = N // ncores             # 12500
        self.pad_rows = 16                   # zero rows appended per stripe
        # 128-aligned quarter sizes summing to shard
        base = (self.shard // (128 * Q)) * 128
        qs = [base] * Q                      # 3072 each, remainder to spread
        rem = self.shard - base * Q          # 212... pad to 128 multiples
        i = 0
        while rem >= 128:
            qs[i] += 128
            rem -= 128
            i = (i + 1) % Q
        qs[-1] += rem                        # last quarter absorbs remainder
        self.qsize = qs                      # e.g. [3200, 3200, 3072, 3028]
        assert sum(qs) == self.shard
        self.qoff = np.concatenate([[0], np.cumsum(qs)]).astype(np.int64)
        self.stripe_rows = [q + self.pad_rows for q in qs]
        self.window_rows = [self.ncores * sr for sr in self.stripe_rows]
        assert max(self.window_rows) <= 32767
        self.tile_cols = tile_cols           # msg tile free columns (rounds)
        self.din, self.dh, self.dout = din, dh, dout
        self.nchunk = (self.shard + 127) // 128
        self.shard_pad = self.nchunk * 128
        # combine waves: one per quarter, sizes padded up to 128 multiples
        self.wsize = [((q + 127) // 128) * 128 for q in self.qsize]
        self.woff = self.qoff[:Q]            # same starts as quarters

    def win_of(self, s):
        """Window (source quarter) of global source id array s."""
        return np.searchsorted(self.qoff, s % self.shard, side="right") - 1

    def src_local(self, s, q):
        """Window-local table row of global source id array s in window q."""
        rank = s // self.shard
        return rank * self.stripe_rows[q] + (s % self.shard - self.qoff[q])

    def zlocal(self, q):
        return self.qsize[q]                 # first zero row of rank 0 stripe


CFG = Cfg()


# ---------------------------------------------------------------- plan

def _wrap16(a):
    """Device idx layout: logical position i lives at [i % 16, i // 16];
    the 16-partition pattern is replicated across all 128 partitions
    (one copy per Q7 core)."""
    a = np.asarray(a, dtype=np.int16)
    assert a.size % 16 == 0
    w = np.ascontiguousarray(a.reshape(-1, 16).T)
    return np.ascontiguousarray(np.tile(w, (8, 1)))


class PassPlan:
    """Shared (cross-core) schedule + per-core index tensors for one
    (window) pass. The same schedule is reused by both layers."""
    __slots__ = ("q", "nblk", "R", "base", "S", "groups", "tiles",
                 "gidx")


class PlanSet(list):
    """List of PassPlan plus the combine-gather plan.

    Per-pass partial sums (ro blocks) are bulk-written to two DRAM "rod"
    tensors (passes 0+1 and 2+3 concatenated, plus a zero row block).
    The combine step gathers, per destination row, its per-pass partials:
    pgidx[k][h][c] holds, for wave k (quarter rows, padded to 128) and
    half h (passes 2h, 2h+1), the rod row of each (pass, dst) partial.
    rod row of rank j in pass p = pass_base + (j % 128) * nblk_p + j // 128.
    """
    __slots__ = ("pgidx", "rod_rows", "zrow")


def build_plan(edge_index, cfg: Cfg):
    """edge_index: [2, E] int array (sources row 0, destinations row 1).
    Returns (plan_list, deg) where plan_list has cfg.Q PassPlan entries."""
    src = np.asarray(edge_index[0], dtype=np.int64)
    dst = np.asarray(edge_index[1], dtype=np.int64)
    N, Q, ncores, shard = cfg.N, cfg.Q, cfg.ncores, cfg.shard

    deg = np.bincount(dst, minlength=N).astype(np.int64) + 1  # + self loop

    # Per (core, window) edge sets.
    core_of = dst // shard
    win_of = cfg.win_of(src)
    # order edges by (window, core) once
    order = np.lexsort((dst, core_of, win_of))
    src_s, dst_s = src[order], dst[order]
    wc_key = win_of[order] * ncores + core_of[order]
    seg_bounds = np.searchsorted(wc_key, np.arange(Q * ncores + 1))

    plans = []
    rank_pos = []   # rank_pos[q][c]: [shard] -> rank j in pass q's order, -1
    for q in range(Q):
        # per-core data for this window
        per_core = []
        for c in range(ncores):
            lo, hi = seg_bounds[q * ncores + c], seg_bounds[q * ncores + c + 1]
            s_loc = cfg.src_local(src_s[lo:hi], q)  # window-local table rows
            d = dst_s[lo:hi]                      # sorted by dst already
            uniq, counts = np.unique(d, return_counts=True)
            # sort destinations by count desc (stable for determinism)
            o = np.argsort(-counts, kind="stable")
            uniq, counts = uniq[o], counts[o]
            per_core.append((s_loc, d, uniq, counts))

        nblk = max((len(u) + 127) // 128 for (_, _, u, _) in per_core)
        nblk = max(nblk, 1)
        Rs = np.zeros(nblk, dtype=np.int64)
        for (_, _, uniq, counts) in per_core:
            nb = (len(uniq) + 127) // 128
            for j in range(nb):
                Rs[j] = max(Rs[j], counts[j * 128])
        Rs = np.maximum(Rs, 1)

        pp = PassPlan()
        pp.q = q
        pp.nblk = nblk
        pp.R = Rs
        pp.base = np.concatenate([[0], np.cumsum(Rs * 128)])
        pp.S = int(pp.base[-1])

        # merge equal-R runs into reduce groups, split into msg tiles
        tiles = []   # list of (cols, [(blk0, nb, R, col0_in_tile), ...])
        cur_groups, cur_cols = [], 0
        j = 0
        while j < nblk:
            r = Rs[j]
            nb_run = 1
            while j + nb_run < nblk and Rs[j + nb_run] == r:
                nb_run += 1
            # split run over tiles
            taken = 0
            while taken < nb_run:
                room = (cfg.tile_cols - cur_cols) // r
                if room <= 0:
                    tiles.append((cur_cols, cur_groups))
                    cur_groups, cur_cols = [], 0
                    room = cfg.tile_cols // r
                    assert room > 0, f"R={r} exceeds tile_cols={cfg.tile_cols}"
                nb_t = min(room, nb_run - taken)
                cur_groups.append((j + taken, nb_t, int(r), cur_cols))
                cur_cols += nb_t * int(r)
                taken += nb_t
            j += nb_run
        if cur_groups:
            tiles.append((cur_cols, cur_groups))
        pp.groups = None
        pp.tiles = tiles

        # per-core index tensors
        pp.gidx = []
        rank_pos.append([])
        for c in range(ncores):
            s_loc, d, uniq, counts = per_core[c]
            gi = np.full(pp.S, cfg.zlocal(q), dtype=np.int64)  # zero row default
            rp = np.full(shard, -1, dtype=np.int64)
            if len(uniq):
                rp[uniq - c * shard] = np.arange(len(uniq))
                rnk = rp[d - c * shard]
                # r index within each destination: edges sorted by dst; order
                # them by rank (stable) so positions within a rank are 0..cnt-1
                o2 = np.argsort(rnk, kind="stable")
                rnk_o = rnk[o2]
                s_o = s_loc[o2]
                starts = np.searchsorted(rnk_o, np.arange(len(uniq)))
                rwithin = np.arange(len(rnk_o)) - starts[rnk_o]
                blk = rnk_o // 128
                lane = rnk_o % 128
                slot = pp.base[blk] + rwithin * 128 + lane
                gi[slot] = s_o
            pp.gidx.append(_wrap16(gi))
            rank_pos[q].append(rp)
        plans.append(pp)

    # ---- combine-gather plan (replaces scatter_add)
    ps = PlanSet(plans)
    ps.rod_rows = []
    ps.zrow = []
    for h in range(2):
        nba, nbb = plans[2 * h].nblk, plans[2 * h + 1].nblk
        ps.zrow.append(128 * (nba + nbb))
        ps.rod_rows.append(128 * (nba + nbb) + 16)
        assert ps.zrow[h] <= 32767
    ps.pgidx = []
    for k in range(Q):
        lo = int(cfg.woff[k])
        wk = cfg.wsize[k]
        halves = []
        for h in range(2):
            pa, pb = 2 * h, 2 * h + 1
            nba, nbb = plans[pa].nblk, plans[pb].nblk
            Z = ps.zrow[h]
            per_core_idx = []
            for c in range(ncores):
                idx = np.full(2 * wk, Z, dtype=np.int64)
                rows = np.arange(lo, min(lo + wk, shard))
                for pl, (p, nb, base) in enumerate(
                        [(pa, nba, 0), (pb, nbb, 128 * nba)]):
                    j = rank_pos[p][c][rows]
                    v = np.where(j >= 0, base + (j % 128) * nb + j // 128, Z)
                    idx[pl * wk:pl * wk + len(rows)] = v
                per_core_idx.append(_wrap16(idx))
            halves.append(per_core_idx)
        ps.pgidx.append(halves)
    return ps, deg


# ---------------------------------------------------------------- numpy golden
# (mirrors device semantics exactly; used for development/testing)

def golden(inputs, cfg: Cfg = CFG):
    x = np.asarray(inputs["x"], np.float32)
    ei = np.asarray(inputs["edge_index"])
    W1 = np.asarray(inputs["W1"], np.float32)
    b1 = np.asarray(inputs["b1"], np.float32)
    W2 = np.asarray(inputs["W2"], np.float32)
    b2 = np.asarray(inputs["b2"], np.float32)
    plans, deg = build_plan(ei, cfg)
    dinv = (1.0 / np.sqrt(deg)).astype(np.float32)

    def windowed(tab, q):
        # [N, d] -> [window_rows[q], d] for window q (source quarter q of
        # every rank, each stripe padded with zero rows)
        d = tab.shape[1]
        t = np.zeros((cfg.ncores, cfg.stripe_rows[q], d), np.float32)
        lo, hi = cfg.qoff[q], cfg.qoff[q + 1]
        t[:, :cfg.qsize[q]] = tab.reshape(cfg.ncores, cfg.shard, d)[:, lo:hi]
        return t.reshape(cfg.window_rows[q], d)

    def propagate(table_full):
        """table_full: [N, d] scaled source features. Returns [N, d] sums of
        incoming messages + self term. Mirrors the device rod/combine path."""
        d_feat = table_full.shape[1]
        out = np.zeros((cfg.N, d_feat), np.float32)
        for c in range(cfg.ncores):
            # per-pass partial blocks -> rod arrays
            rods = [np.zeros((plans.rod_rows[h], d_feat), np.float32)
                    for h in range(2)]
            for pp in plans:
                tabw = windowed(table_full, pp.q)
                gi = pp.gidx[c][:16].T.reshape(-1)      # unwrap
                msg = tabw[gi]                     # [S, d]
                h, sub = pp.q // 2, pp.q % 2
                base = 0 if sub == 0 else 128 * plans[2 * h].nblk
                nb = pp.nblk
                lanes = np.arange(128) * nb
                for j in range(pp.nblk):
                    r = int(pp.R[j])
                    seg = msg[pp.base[j]:pp.base[j + 1]].reshape(
                        r, 128, d_feat).sum(0)       # [128 lanes, d]
                    rods[h][base + lanes + j] = seg
            # combine: self + gathered per-pass partials
            a = table_full[c * cfg.shard:(c + 1) * cfg.shard].copy()
            for k in range(cfg.Q):
                lo = int(cfg.woff[k])
                wk = cfg.wsize[k]
                nrows = min(lo + wk, cfg.shard) - lo
                for h in range(2):
                    idx = plans.pgidx[k][h][c][:16].T.reshape(-1)
                    vals = rods[h][idx]              # [2*wk, d]
                    part = vals[:wk] + vals[wk:]
                    a[lo:lo + nrows] += part[:nrows]
            out[c * cfg.shard:(c + 1) * cfg.shard] = a
        return out

    hpp = (x @ W1) * dinv[:, None]
    acc1 = propagate(hpp)
    hid = acc1 * dinv[:, None] + b1
    hid = np.where(hid > 0, hid, 0.01 * hid)
    hpp2 = hid * dinv[:, None]
    acc2 = propagate(hpp2)
    return (acc2 * dinv[:, None]) @ W2 + b2


# ---------------------------------------------------------------- bass program

def build_bass(plans, cfg: Cfg, debug=False):
    import concourse.bass as bass
    import concourse.mybir as mybir
    import concourse.tile as tile
    from concourse import bacc
    from concourse.masks import make_identity

    f32 = mybir.dt.float32
    i16 = mybir.dt.int16
    P = 128
    shard, Q, nchunk = cfg.shard, cfg.Q, cfg.nchunk
    dh, dout = cfg.dh, cfg.dout

    nc = bacc.Bacc(None, target_bir_lowering=False, debug=debug,
                   num_swdge_queues=4, dynamic_dma_scratch_size=32768)

    # ---- external I/O (per-core shapes; SPMD-uniform)
    xT = nc.declare_dram_parameter("xT", [cfg.din, shard], f32, isOutput=False)
    W1p = nc.declare_dram_parameter("W1", [cfg.din, dh], f32, isOutput=False)
    b1p = nc.declare_dram_parameter("b1", [1, dh], f32, isOutput=False)
    W2p = nc.declare_dram_parameter("W2", [dh, dout], f32, isOutput=False)
    b2p = nc.declare_dram_parameter("b2", [1, dout], f32, isOutput=False)
    dinvp = nc.declare_dram_parameter("dinv_col", [P, nchunk], f32, isOutput=False)
    gidxp = [nc.declare_dram_parameter(f"gidx_p{q}", list(plans[q].gidx[0].shape),
                                       i16, isOutput=False) for q in range(Q)]
    pgidxp = [[nc.declare_dram_parameter(
        f"pgidx_k{k}h{h}", list(plans.pgidx[k][h][0].shape), i16,
        isOutput=False) for h in range(2)] for k in range(Q)]
    outp = nc.declare_dram_parameter("out", [shard, dout], f32, isOutput=True)

    # ---- internal DRAM (per-layer, per-quarter-window)
    ag_in = [[nc.dram_tensor(f"ag_in{l}_{q}", [cfg.stripe_rows[q], dh], f32)
              for q in range(Q)] for l in (0, 1)]
    table = [[nc.dram_tensor(f"table{l}_{q}",
                             [cfg.ncores * cfg.stripe_rows[q], dh], f32,
                             addr_space="Shared") for q in range(Q)]
             for l in (0, 1)]
    # per-pass partial blocks (passes 0+1 / 2+3 concatenated + zero rows);
    # reused by both layers
    rod = [nc.dram_tensor(f"rod{h}", [plans.rod_rows[h], dh], f32)
           for h in range(2)]

    core_ids = list(range(cfg.ncores))

    # chunk j (rows 128j..) -> (quarter q, row offset within quarter);
    # quarter boundaries are 128-aligned except the final end.
    def chunk_quarter(j):
        row0 = j * 128
        q = int(np.searchsorted(cfg.qoff, row0, side="right") - 1)
        return q, row0 - int(cfg.qoff[q])

    qend_chunk = [int((cfg.qoff[q + 1] - 1) // 128) for q in range(Q)]

    with tile.TileContext(nc) as tc:
        with (
            tc.tile_pool(name="const", bufs=1) as constp,
            tc.tile_pool(name="big", bufs=1) as bigp,
            tc.tile_pool(name="chunk", bufs=3) as chp,
            tc.tile_pool(name="msg", bufs=2) as msgp,
            tc.tile_pool(name="rout", bufs=2) as routp,
            tc.tile_pool(name="cmb", bufs=1) as cmbp,
            tc.tile_pool(name="psum", bufs=2, space="PSUM") as psp,
            tc.tile_pool(name="psum1", bufs=2, space="PSUM") as psp1,
        ):
            # ------- constants
            w1_s = constp.tile([cfg.din, dh], f32)
            nc.sync.dma_start(w1_s[:], W1p[:])
            w2_s = constp.tile([dh, dout], f32)
            nc.sync.dma_start(w2_s[:], W2p[:])
            b1_s = constp.tile([P, dh], f32)
            nc.sync.dma_start(b1_s[:], b1p[:1, :].to_broadcast((P, dh)))
            b2_s = constp.tile([P, dout], f32)
            nc.sync.dma_start(b2_s[:], b2p[:1, :].to_broadcast((P, dout)))
            dinv_s = constp.tile([P, nchunk], f32)
            nc.sync.dma_start(dinv_s[:], dinvp[:])
            ident = constp.tile([P, P], f32)
            make_identity(nc, ident[:])
            zpad = constp.tile([cfg.pad_rows, dh], f32)
            nc.vector.memset(zpad[:], 0.0)
            for l in (0, 1):
                for q in range(Q):
                    nc.sync.dma_start(
                        ag_in[l][q][cfg.qsize[q]:cfg.stripe_rows[q], :],
                        zpad[:])

            # gather/combine index tiles (resident; reused by both layers)
            gidx_s = []
            pgidx_s = []
            for q in range(Q):
                g = constp.tile(list(plans[q].gidx[0].shape), i16, tag=f"gidx{q}")
                nc.sync.dma_start(g[:], gidxp[q][:])
                gidx_s.append(g)
                hs = []
                for h in range(2):
                    s = constp.tile(list(plans.pgidx[q][h][0].shape), i16,
                                    tag=f"pgidx{q}_{h}")
                    nc.sync.dma_start(s[:], pgidxp[q][h][:])
                    hs.append(s)
                pgidx_s.append(hs)
            # zero the rod zero-row blocks (tail 16 rows; layer-shared)
            for h in range(2):
                nc.sync.dma_start(rod[h][plans.zrow[h]:plans.rod_rows[h], :],
                                  zpad[:])

            def allgather(l, q):
                nc.gpsimd.collective_compute(
                    "AllGather", mybir.AluOpType.bypass,
                    replica_groups=[core_ids],
                    ins=[ag_in[l][q][:].opt()],
                    outs=[table[l][q][:].opt()],
                )

            # ------- phase A: h'' = (x @ W1) * dinv, write ag_in0 + acc0 prefill
            # self terms stay resident in SBUF (h'' for layer 1, hid'' for 2)
            hc_all = bigp.tile([P, nchunk, dh], f32, tag="hc_all")
            hid_all = bigp.tile([P, nchunk, dh], f32, tag="hid_all")

            for j in range(nchunk):
                nj = min(P, shard - j * P)
                qj, off = chunk_quarter(j)
                xc = chp.tile([cfg.din, P], f32, tag="xc")
                nc.sync.dma_start(xc[:, :nj], xT[:, j * P:j * P + nj])
                ph = psp.tile([P, dh], f32, tag="mm")
                nc.tensor.matmul(ph[:nj, :], lhsT=xc[:, :nj],
                                 rhs=w1_s[:], start=True, stop=True)
                nc.vector.tensor_scalar_mul(hc_all[:nj, j, :], ph[:nj, :],
                                            dinv_s[:nj, j:j + 1])
                nc.sync.dma_start(ag_in[0][qj][off:off + nj, :],
                                  hc_all[:nj, j, :])
                if j == qend_chunk[qj]:
                    allgather(0, qj)

            qctr = [0]

            def next_q():
                q = qctr[0] % 4
                qctr[0] += 1
                return q

            def do_passes(l):
                for pp in plans:
                    tab = table[l][pp.q]
                    h, sub = pp.q // 2, pp.q % 2
                    rbase = 0 if sub == 0 else 128 * plans[2 * h].nblk
                    rodv = rod[h][rbase:rbase + P * pp.nblk, :] \
                        .rearrange("(l b) f -> l b f", l=P)
                    col_cum = 0
                    for (cols, groups) in pp.tiles:
                        mt = msgp.tile([P, cfg.tile_cols, dh], f32, tag="mt")
                        nidx = cols * P
                        slot0 = col_cum * P
                        nc.gpsimd.dma_gather(
                            mt[:, :cols, :],
                            tab[:, :],
                            gidx_s[pp.q][:, slot0 // 16:(slot0 + nidx) // 16],
                            nidx, nidx, dh, single_packet=False,
                            queue_num=next_q(),
                        )
                        blk_lo = groups[0][0]
                        blk_hi = groups[-1][0] + groups[-1][1]
                        rog = routp.tile([P, cfg.tile_cols, dh], f32,
                                         tag="rog")
                        for (blk0, nb, r, col0) in groups:
                            seg = mt[:, col0:col0 + nb * r, :]
                            seg = seg.rearrange("p (b r) f -> p b f r", r=r)
                            nc.vector.reduce_sum(
                                rog[:, blk0 - blk_lo:blk0 - blk_lo + nb, :],
                                seg, axis=mybir.AxisListType.X)
                        # write this tile's partial blocks (row = lane*nblk+b)
                        nc.sync.dma_start(
                            rodv[:, blk_lo:blk_hi, :],
                            rog[:, :blk_hi - blk_lo, :])
                        col_cum += cols

            def combine_wave(k, self_all, body):
                """Pre-gather wave k's per-pass partials, then run body(j, t)
                for each chunk j of the wave, with t = self + sum of partials
                for chunk j's rows."""
                lo = int(cfg.woff[k])
                wk = cfg.wsize[k]
                wc = wk // P
                wcm = max(w // P for w in cfg.wsize)
                parts = []
                for h in range(2):
                    pg = cmbp.tile([P, 2 * wcm, dh], f32, tag=f"pg{h}")
                    nc.gpsimd.dma_gather(
                        pg[:, :2 * wc, :], rod[h][:, :], pgidx_s[k][h][:],
                        2 * wk, 2 * wk, dh, single_packet=False,
                        queue_num=next_q(),
                    )
                    red = cmbp.tile([P, wcm, dh], f32, tag=f"pr{h}")
                    nc.vector.reduce_sum(
                        red[:, :wc, :],
                        pg[:, :2 * wc, :].rearrange("p (a c) f -> p c f a",
                                                    a=2),
                        axis=mybir.AxisListType.X)
                    parts.append(red)
                part = cmbp.tile([P, wcm, dh], f32, tag="part")
                nc.vector.tensor_add(part[:, :wc, :], parts[0][:, :wc, :],
                                     parts[1][:, :wc, :])
                j0 = lo // P
                for cj in range(wc):
                    j = j0 + cj
                    if j >= nchunk:
                        break
                    nj = min(P, shard - j * P)
                    t = chp.tile([P, dh], f32, tag="t")
                    nc.vector.tensor_add(t[:nj, :], self_all[:nj, j, :],
                                         part[:nj, cj, :])
                    body(j, nj, t)

            do_passes(0)

            # ------- phase D: hid'' = leakyrelu(acc0*dinv + b1) * dinv
            def phase_d_body(j, nj, t):
                qj, off = chunk_quarter(j)
                t0 = chp.tile([P, dh], f32, tag="t0")
                nc.vector.tensor_scalar_mul(t0[:nj, :], t[:nj, :],
                                            dinv_s[:nj, j:j + 1])
                nc.vector.tensor_add(t0[:nj, :], t0[:nj, :], b1_s[:nj, :])
                t1 = chp.tile([P, dh], f32, tag="t1")
                nc.vector.tensor_scalar_mul(t1[:nj, :], t0[:nj, :], 0.01)
                nc.vector.tensor_max(t0[:nj, :], t0[:nj, :], t1[:nj, :])
                nc.vector.tensor_scalar_mul(hid_all[:nj, j, :], t0[:nj, :],
                                            dinv_s[:nj, j:j + 1])
                nc.sync.dma_start(ag_in[1][qj][off:off + nj, :],
                                  hid_all[:nj, j, :])
                if j == qend_chunk[qj]:
                    allgather(1, qj)

            for k in range(Q):
                combine_wave(k, hc_all, phase_d_body)

            do_passes(1)

            # ------- phase G: out = (acc1*dinv) @ W2 + b2
            def phase_g_body(j, nj, t):
                p2 = chp.tile([P, dh], f32, tag="p2")
                nc.vector.tensor_scalar_mul(p2[:nj, :], t[:nj, :],
                                            dinv_s[:nj, j:j + 1])
                ptr = psp.tile([dh, P], f32, tag="tr")
                nc.tensor.transpose(ptr[:, :nj], p2[:nj, :], ident[:nj, :nj])
                p2t = chp.tile([dh, P], f32, tag="p2t")
                nc.vector.tensor_copy(p2t[:, :nj], ptr[:, :nj])
                po = psp1.tile([P, dout], f32, tag="mo")
                nc.tensor.matmul(po[:nj, :], lhsT=p2t[:, :nj], rhs=w2_s[:],
                                 start=True, stop=True)
                oc = chp.tile([P, dout], f32, tag="oc")
                nc.vector.tensor_add(oc[:nj, :], po[:nj, :], b2_s[:nj, :])
                nc.sync.dma_start(outp[j * P:j * P + nj, :], oc[:nj, :])

            for k in range(Q):
                combine_wave(k, hid_all, phase_g_body)

    nc.compile()
    return nc


# ---------------------------------------------------------------- inputs per core

def make_in_maps(inputs, plans, cfg: Cfg):
    x = np.ascontiguousarray(np.asarray(inputs["x"], np.float32))
    W1 = np.ascontiguousarray(np.asarray(inputs["W1"], np.float32))
    b1 = np.ascontiguousarray(np.asarray(inputs["b1"], np.float32).reshape(1, -1))
    W2 = np.ascontiguousarray(np.asarray(inputs["W2"], np.float32))
    b2 = np.ascontiguousarray(np.asarray(inputs["b2"], np.float32).reshape(1, -1))
    _, deg = None, None  # deg recomputed below to avoid threading state
    dst = np.asarray(inputs["edge_index"][1], dtype=np.int64)
    degv = np.bincount(dst, minlength=cfg.N).astype(np.int64) + 1
    dinv = (1.0 / np.sqrt(degv)).astype(np.float32)

    in_maps = []
    for c in range(cfg.ncores):
        sl = slice(c * cfg.shard, (c + 1) * cfg.shard)
        xTc = np.ascontiguousarray(x[sl].T)
        dpad = np.zeros(cfg.shard_pad, np.float32)
        dpad[:cfg.shard] = dinv[sl]
        dcol = np.ascontiguousarray(dpad.reshape(cfg.nchunk, 128).T)
        m = {"xT": xTc, "W1": W1, "b1": b1, "W2": W2, "b2": b2,
             "dinv_col": dcol}
        for q in range(cfg.Q):
            m[f"gidx_p{q}"] = plans[q].gidx[c]
            for h in range(2):
                m[f"pgidx_k{q}h{h}"] = plans.pgidx[q][h][c]
        in_maps.append(m)
    return in_maps


# ---------------------------------------------------------------- entry point

def kernel(**inputs):
    from concourse.bass_utils import run_bass_kernel_spmd
    cfg = CFG
    plans, _deg = build_plan(inputs["edge_index"], cfg)
    nc = build_bass(plans, cfg)
    in_maps = make_in_maps(inputs, plans, cfg)
    core_ids = list(range(cfg.ncores))
    res = run_bass_kernel_spmd(nc, in_maps, core_ids).results
    out = np.concatenate([res[c]["out"] for c in core_ids], axis=0)
    return out.astype(np.float32)



# revision 32
# speedup vs baseline: 1.2390x; 1.2390x over previous
"""GCN 2-layer (PyG GCNConv x2) Trainium2 kernel, 8-core SPMD.

Strategy:
  - Shard destination nodes across the 8 cores (12500 each). Weights
    replicated. SWDGE descriptor generation is the machine bottleneck, so
    gathers/scatter-free combines round-robin over 4 SWDGE queues and the
    descriptor count is minimized.
  - Windows are source QUARTERS of every rank so the per-layer AllGather
    runs as 4 pipelined quarter-collectives overlapping phase compute and
    the first gather passes.
  - Layer l: each core computes its shard of the scaled features
    (h'' = (x @ W) * dinv for layer 1, hid'' = leakyrelu(...)*dinv for 2),
    kept resident in SBUF (self-loop term) and quarter-AllGathered into
    per-window tables in HBM.
  - Message gather: MoE dma_gather (int16 idxs) from the window table.
    Per (core, window) pass, destinations are sorted by in-window degree
    and laid out on the 128 SBUF partitions; each block of 128 destinations
    has a uniform round count R (max degree in the block); gathered
    messages land [128 lanes, R rounds, 64] and a strided reduce_sum
    collapses R into per-block partial sums.
  - Partial blocks are bulk-written (HWDGE, no descriptors on the SWDGE
    path) to per-pass-pair "rod" DRAM tensors. The combine step then
    gathers, per destination row, its 4 per-pass partials (one descriptor
    per row-pass instead of a 2-descriptor RMW scatter per row) and sums
    them with the SBUF-resident self term — fused into phase D (layer
    boundary) and phase G (output).
  - Layer 2 reuses the identical edge schedule/index tensors on the hid''
    table; final output = (acc2 * dinv) @ W2 + b2 via PE transpose+matmul.
"""

import numpy as np


# ---------------------------------------------------------------- config

class Cfg:
    """Windows are source-QUARTERS of every rank (quarter q of each shard),
    so AllGather can run as 4 pipelined quarter-collectives.

    Window q's table = concat over ranks of [qsize[q] rows + pad zeros].
    """
    def __init__(self, N=100000, E=1200000, ncores=8, Q=4, tile_cols=32,
                 din=128, dh=64, dout=40):
        assert N % ncores == 0
        self.N, self.E, self.ncores, self.Q = N, E, ncores, Q
        self.shard = N // ncores             # 12500
        self.pad_rows = 16                   # zero rows appended per stripe
        # 128-aligned quarter sizes summing to shard
        base = (self.shard // (128 * Q)) * 128
        qs = [base] * Q                      # 3072 each, remainder to spread
        rem = self.shard - base * Q          # 212... pad to 128 multiples
        i = 0
        while rem >= 128:
            qs[i] += 128
            rem -= 128
            i = (i + 1) % Q
        qs[-1] += rem                        # last quarter absorbs remainder
        self.qsize = qs                      # e.g. [3200, 3200, 3072, 3028]
        assert sum(qs) == self.shard
        self.qoff = np.concatenate([[0], np.cumsum(qs)]).astype(np.int64)
        self.stripe_rows = [q + self.pad_rows for q in qs]
        self.window_rows = [self.ncores * sr for sr in self.stripe_rows]
        assert max(self.window_rows) <= 32767
        self.tile_cols = tile_cols           # msg tile free columns (rounds)
        self.din, self.dh, self.dout = din, dh, dout
        self.nchunk = (self.shard + 127) // 128
        self.shard_pad = self.nchunk * 128
        # combine waves: one per quarter, sizes padded up to 128 multiples
        self.wsize = [((q + 127) // 128) * 128 for q in self.qsize]
        self.woff = self.qoff[:Q]            # same starts as quarters

    def win_of(self, s):
        """Window (source quarter) of global source id array s."""
        return np.searchsorted(self.qoff, s % self.shard, side="right") - 1

    def src_local(self, s, q):
        """Window-local table row of global source id array s in window q."""
        rank = s // self.shard
        return rank * self.stripe_rows[q] + (s % self.shard - self.qoff[q])

    def zlocal(self, q):
        return self.qsize[q]                 # first zero row of rank 0 stripe


CFG = Cfg()


# ---------------------------------------------------------------- plan

def _wrap16(a):
    """Device idx layout: logical position i lives at [i % 16, i // 16];
    the 16-partition pattern is replicated across all 128 partitions
    (one copy per Q7 core)."""
    a = np.asarray(a, dtype=np.int16)
    assert a.size % 16 == 0
    w = np.ascontiguousarray(a.reshape(-1, 16).T)
    return np.ascontiguousarray(np.tile(w, (8, 1)))


class PassPlan:
    """Shared (cross-core) schedule + per-core index tensors for one
    (window) pass. The same schedule is reused by both layers."""
    __slots__ = ("q", "nblk", "R", "base", "S", "groups", "tiles",
                 "gidx")


class PlanSet(list):
    """List of PassPlan plus the combine-gather plan.

    Per-pass partial sums (ro blocks) are bulk-written to two DRAM "rod"
    tensors (passes 0+1 and 2+3 concatenated, plus a zero row block).
    The combine step gathers, per destination row, its per-pass partials:
    pgidx[k][h][c] holds, for wave k (quarter rows, padded to 128) and
    half h (passes 2h, 2h+1), the rod row of each (pass, dst) partial.
    rod row of rank j in pass p = pass_base + (j % 128) * nblk_p + j // 128.
    """
    __slots__ = ("pgidx", "rod_rows", "zrow")


def build_plan(edge_index, cfg: Cfg):
    """edge_index: [2, E] int array (sources row 0, destinations row 1).
    Returns (plan_list, deg) where plan_list has cfg.Q PassPlan entries."""
    src = np.asarray(edge_index[0], dtype=np.int64)
    dst = np.asarray(edge_index[1], dtype=np.int64)
    N, Q, ncores, shard = cfg.N, cfg.Q, cfg.ncores, cfg.shard

    deg = np.bincount(dst, minlength=N).astype(np.int64) + 1  # + self loop

    # Per (core, window) edge sets.
    core_of = dst // shard
    win_of = cfg.win_of(src)
    # order edges by (window, core) once
    order = np.lexsort((dst, core_of, win_of))
    src_s, dst_s = src[order], dst[order]
    wc_key = win_of[order] * ncores + core_of[order]
    seg_bounds = np.searchsorted(wc_key, np.arange(Q * ncores + 1))

    plans = []
    rank_pos = []   # rank_pos[q][c]: [shard] -> rank j in pass q's order, -1
    for q in range(Q):
        # per-core data for this window
        per_core = []
        for c in range(ncores):
            lo, hi = seg_bounds[q * ncores + c], seg_bounds[q * ncores + c + 1]
            s_loc = cfg.src_local(src_s[lo:hi], q)  # window-local table rows
            d = dst_s[lo:hi]                      # sorted by dst already
            uniq, counts = np.unique(d, return_counts=True)
            # sort destinations by count desc (stable for determinism)
            o = np.argsort(-counts, kind="stable")
            uniq, counts = uniq[o], counts[o]
            per_core.append((s_loc, d, uniq, counts))

        nblk = max((len(u) + 127) // 128 for (_, _, u, _) in per_core)
        nblk = max(nblk, 1)
        Rs = np.zeros(nblk, dtype=np.int64)
        for (_, _, uniq, counts) in per_core:
            nb = (len(uniq) + 127) // 128
            for j in range(nb):
                Rs[j] = max(Rs[j], counts[j * 128])
        Rs = np.maximum(Rs, 1)

        pp = PassPlan()
        pp.q = q
        pp.nblk = nblk
        pp.R = Rs
        pp.base = np.concatenate([[0], np.cumsum(Rs * 128)])
        pp.S = int(pp.base[-1])

        # merge equal-R runs into reduce groups, split into msg tiles
        tiles = []   # list of (cols, [(blk0, nb, R, col0_in_tile), ...])
        cur_groups, cur_cols = [], 0
        j = 0
        while j < nblk:
            r = Rs[j]
            nb_run = 1
            while j + nb_run < nblk and Rs[j + nb_run] == r:
                nb_run += 1
            # split run over tiles
            taken = 0
            while taken < nb_run:
                room = (cfg.tile_cols - cur_cols) // r
                if room <= 0:
                    tiles.append((cur_cols, cur_groups))
                    cur_groups, cur_cols = [], 0
                    room = cfg.tile_cols // r
                    assert room > 0, f"R={r} exceeds tile_cols={cfg.tile_cols}"
                nb_t = min(room, nb_run - taken)
                cur_groups.append((j + taken, nb_t, int(r), cur_cols))
                cur_cols += nb_t * int(r)
                taken += nb_t
            j += nb_run
        if cur_groups:
            tiles.append((cur_cols, cur_groups))
        pp.groups = None
        pp.tiles = tiles

        # per-core index tensors
        pp.gidx = []
        rank_pos.append([])
        for c in range(ncores):
            s_loc, d, uniq, counts = per_core[c]
            gi = np.full(pp.S, cfg.zlocal(q), dtype=np.int64)  # zero row default
            rp = np.full(shard, -1, dtype=np.int64)
            if len(uniq):
                rp[uniq - c * shard] = np.arange(len(uniq))
                rnk = rp[d - c * shard]
                # r index within each destination: edges sorted by dst; order
                # them by rank (stable) so positions within a rank are 0..cnt-1
                o2 = np.argsort(rnk, kind="stable")
                rnk_o = rnk[o2]
                s_o = s_loc[o2]
                starts = np.searchsorted(rnk_o, np.arange(len(uniq)))
                rwithin = np.arange(len(rnk_o)) - starts[rnk_o]
                blk = rnk_o // 128
                lane = rnk_o % 128
                slot = pp.base[blk] + rwithin * 128 + lane
                gi[slot] = s_o
            pp.gidx.append(_wrap16(gi))
            rank_pos[q].append(rp)
        plans.append(pp)

    # ---- combine-gather plan (replaces scatter_add)
    ps = PlanSet(plans)
    ps.rod_rows = []
    ps.zrow = []
    for h in range(2):
        nba, nbb = plans[2 * h].nblk, plans[2 * h + 1].nblk
        ps.zrow.append(128 * (nba + nbb))
        ps.rod_rows.append(128 * (nba + nbb) + 16)
        assert ps.zrow[h] <= 32767
    ps.pgidx = []
    for k in range(Q):
        lo = int(cfg.woff[k])
        wk = cfg.wsize[k]
        halves = []
        for h in range(2):
            pa, pb = 2 * h, 2 * h + 1
            nba, nbb = plans[pa].nblk, plans[pb].nblk
            Z = ps.zrow[h]
            per_core_idx = []
            for c in range(ncores):
                idx = np.full(2 * wk, Z, dtype=np.int64)
                rows = np.arange(lo, min(lo + wk, shard))
                for pl, (p, nb, base) in enumerate(
                        [(pa, nba, 0), (pb, nbb, 128 * nba)]):
                    j = rank_pos[p][c][rows]
                    v = np.where(j >= 0, base + (j % 128) * nb + j // 128, Z)
                    idx[pl * wk:pl * wk + len(rows)] = v
                per_core_idx.append(_wrap16(idx))
            halves.append(per_core_idx)
        ps.pgidx.append(halves)
    return ps, deg


# ---------------------------------------------------------------- numpy golden
# (mirrors device semantics exactly; used for development/testing)

def golden(inputs, cfg: Cfg = CFG):
    x = np.asarray(inputs["x"], np.float32)
    ei = np.asarray(inputs["edge_index"])
    W1 = np.asarray(inputs["W1"], np.float32)
    b1 = np.asarray(inputs["b1"], np.float32)
    W2 = np.asarray(inputs["W2"], np.float32)
    b2 = np.asarray(inputs["b2"], np.float32)
    plans, deg = build_plan(ei, cfg)
    dinv = (1.0 / np.sqrt(deg)).astype(np.float32)

    def windowed(tab, q):
        # [N, d] -> [window_rows[q], d] for window q (source quarter q of
        # every rank, each stripe padded with zero rows)
        d = tab.shape[1]
        t = np.zeros((cfg.ncores, cfg.stripe_rows[q], d), np.float32)
        lo, hi = cfg.qoff[q], cfg.qoff[q + 1]
        t[:, :cfg.qsize[q]] = tab.reshape(cfg.ncores, cfg.shard, d)[:, lo:hi]
        return t.reshape(cfg.window_rows[q], d)

    def propagate(table_full):
        """table_full: [N, d] scaled source features. Returns [N, d] sums of
        incoming messages + self term. Mirrors the device rod/combine path."""
        d_feat = table_full.shape[1]
        out = np.zeros((cfg.N, d_feat), np.float32)
        for c in range(cfg.ncores):
            # per-pass partial blocks -> rod arrays
            rods = [np.zeros((plans.rod_rows[h], d_feat), np.float32)
                    for h in range(2)]
            for pp in plans:
                tabw = windowed(table_full, pp.q)
                gi = pp.gidx[c][:16].T.reshape(-1)      # unwrap
                msg = tabw[gi]                     # [S, d]
                h, sub = pp.q // 2, pp.q % 2
                base = 0 if sub == 0 else 128 * plans[2 * h].nblk
                nb = pp.nblk
                lanes = np.arange(128) * nb
                for j in range(pp.nblk):
                    r = int(pp.R[j])
                    seg = msg[pp.base[j]:pp.base[j + 1]].reshape(
                        r, 128, d_feat).sum(0)       # [128 lanes, d]
                    rods[h][base + lanes + j] = seg
            # combine: self + gathered per-pass partials
            a = table_full[c * cfg.shard:(c + 1) * cfg.shard].copy()
            for k in range(cfg.Q):
                lo = int(cfg.woff[k])
                wk = cfg.wsize[k]
                nrows = min(lo + wk, cfg.shard) - lo
                for h in range(2):
                    idx = plans.pgidx[k][h][c][:16].T.reshape(-1)
                    vals = rods[h][idx]              # [2*wk, d]
                    part = vals[:wk] + vals[wk:]
                    a[lo:lo + nrows] += part[:nrows]
            out[c * cfg.shard:(c + 1) * cfg.shard] = a
        return out

    hpp = (x @ W1) * dinv[:, None]
    acc1 = propagate(hpp)
    hid = acc1 * dinv[:, None] + b1
    hid = np.where(hid > 0, hid, 0.01 * hid)
    hpp2 = hid * dinv[:, None]
    acc2 = propagate(hpp2)
    return (acc2 * dinv[:, None]) @ W2 + b2


# ---------------------------------------------------------------- bass program

def build_bass(plans, cfg: Cfg, debug=False):
    import concourse.bass as bass
    import concourse.mybir as mybir
    import concourse.tile as tile
    from concourse import bacc
    from concourse.masks import make_identity

    f32 = mybir.dt.float32
    i16 = mybir.dt.int16
    P = 128
    shard, Q, nchunk = cfg.shard, cfg.Q, cfg.nchunk
    dh, dout = cfg.dh, cfg.dout

    nc = bacc.Bacc(None, target_bir_lowering=False, debug=debug,
                   num_swdge_queues=4, dynamic_dma_scratch_size=32768)

    # ---- external I/O (per-core shapes; SPMD-uniform)
    xT = nc.declare_dram_parameter("xT", [cfg.din, shard], f32, isOutput=False)
    W1p = nc.declare_dram_parameter("W1", [cfg.din, dh], f32, isOutput=False)
    b1p = nc.declare_dram_parameter("b1", [1, dh], f32, isOutput=False)
    W2p = nc.declare_dram_parameter("W2", [dh, dout], f32, isOutput=False)
    b2p = nc.declare_dram_parameter("b2", [1, dout], f32, isOutput=False)
    dinvp = nc.declare_dram_parameter("dinv_col", [P, nchunk], f32, isOutput=False)
    gidxp = [nc.declare_dram_parameter(f"gidx_p{q}", list(plans[q].gidx[0].shape),
                                       i16, isOutput=False) for q in range(Q)]
    pgidxp = [[nc.declare_dram_parameter(
        f"pgidx_k{k}h{h}", list(plans.pgidx[k][h][0].shape), i16,
        isOutput=False) for h in range(2)] for k in range(Q)]
    outp = nc.declare_dram_parameter("out", [shard, dout], f32, isOutput=True)

    # ---- internal DRAM (per-layer, per-quarter-window)
    ag_in = [[nc.dram_tensor(f"ag_in{l}_{q}", [cfg.stripe_rows[q], dh], f32)
              for q in range(Q)] for l in (0, 1)]
    table = [[nc.dram_tensor(f"table{l}_{q}",
                             [cfg.ncores * cfg.stripe_rows[q], dh], f32,
                             addr_space="Shared") for q in range(Q)]
             for l in (0, 1)]
    # per-pass partial blocks (passes 0+1 / 2+3 concatenated + zero rows);
    # reused by both layers
    rod = [nc.dram_tensor(f"rod{h}", [plans.rod_rows[h], dh], f32)
           for h in range(2)]

    core_ids = list(range(cfg.ncores))

    # chunk j (rows 128j..) -> (quarter q, row offset within quarter);
    # quarter boundaries are 128-aligned except the final end.
    def chunk_quarter(j):
        row0 = j * 128
        q = int(np.searchsorted(cfg.qoff, row0, side="right") - 1)
        return q, row0 - int(cfg.qoff[q])

    qend_chunk = [int((cfg.qoff[q + 1] - 1) // 128) for q in range(Q)]

    with tile.TileContext(nc) as tc:
        with (
            tc.tile_pool(name="const", bufs=1) as constp,
            tc.tile_pool(name="big", bufs=1) as bigp,
            tc.tile_pool(name="chunk", bufs=3) as chp,
            tc.tile_pool(name="msg", bufs=2) as msgp,
            tc.tile_pool(name="rout", bufs=2) as routp,
            tc.tile_pool(name="cmb", bufs=1) as cmbp,
            tc.tile_pool(name="psum", bufs=2, space="PSUM") as psp,
            tc.tile_pool(name="psum1", bufs=2, space="PSUM") as psp1,
        ):
            # ------- constants
            w1_s = constp.tile([cfg.din, dh], f32)
            nc.sync.dma_start(w1_s[:], W1p[:])
            w2_s = constp.tile([dh, dout], f32)
            nc.sync.dma_start(w2_s[:], W2p[:])
            b1_s = constp.tile([P, dh], f32)
            nc.sync.dma_start(b1_s[:], b1p[:1, :].to_broadcast((P, dh)))
            b2_s = constp.tile([P, dout], f32)
            nc.sync.dma_start(b2_s[:], b2p[:1, :].to_broadcast((P, dout)))
            dinv_s = constp.tile([P, nchunk], f32)
            nc.sync.dma_start(dinv_s[:], dinvp[:])
            ident = constp.tile([P, P], f32)
            make_identity(nc, ident[:])
            zpad = constp.tile([cfg.pad_rows, dh], f32)
            nc.vector.memset(zpad[:], 0.0)
            for l in (0, 1):
                for q in range(Q):
                    nc.sync.dma_start(
                        ag_in[l][q][cfg.qsize[q]:cfg.stripe_rows[q], :],
                        zpad[:])

            # gather/combine index tiles (resident; reused by both layers)
            gidx_s = []
            pgidx_s = []
            for q in range(Q):
                g = constp.tile(list(plans[q].gidx[0].shape), i16, tag=f"gidx{q}")
                nc.sync.dma_start(g[:], gidxp[q][:])
                gidx_s.append(g)
                hs = []
                for h in range(2):
                    s = constp.tile(list(plans.pgidx[q][h][0].shape), i16,
                                    tag=f"pgidx{q}_{h}")
                    nc.sync.dma_start(s[:], pgidxp[q][h][:])
                    hs.append(s)
                pgidx_s.append(hs)
            # zero the rod zero-row blocks (tail 16 rows; layer-shared)
            for h in range(2):
                nc.sync.dma_start(rod[h][plans.zrow[h]:plans.rod_rows[h], :],
                                  zpad[:])

            def allgather(l, q):
                nc.gpsimd.collective_compute(
                    "AllGather", mybir.AluOpType.bypass,
                    replica_groups=[core_ids],
                    ins=[ag_in[l][q][:].opt()],
                    outs=[table[l][q][:].opt()],
                )

            # ------- phase A: h'' = (x @ W1) * dinv, write ag_in0 + acc0 prefill
            # self terms stay resident in SBUF (h'' for layer 1, hid'' for 2)
            hc_all = bigp.tile([P, nchunk, dh], f32, tag="hc_all")
            hid_all = bigp.tile([P, nchunk, dh], f32, tag="hid_all")

            for j in range(nchunk):
                nj = min(P, shard - j * P)
                qj, off = chunk_quarter(j)
                xc = chp.tile([cfg.din, P], f32, tag="xc")
                nc.sync.dma_start(xc[:, :nj], xT[:, j * P:j * P + nj])
                ph = psp.tile([P, dh], f32, tag="mm")
                nc.tensor.matmul(ph[:nj, :], lhsT=xc[:, :nj],
                                 rhs=w1_s[:], start=True, stop=True)
                nc.vector.tensor_scalar_mul(hc_all[:nj, j, :], ph[:nj, :],
                                            dinv_s[:nj, j:j + 1])
                nc.sync.dma_start(ag_in[0][qj][off:off + nj, :],
                                  hc_all[:nj, j, :])
                if j == qend_chunk[qj]:
                    allgather(0, qj)

            qctr = [0]

            def next_q():
                q = qctr[0] % 4
                qctr[0] += 1
                return q

            def do_passes(l):
                for pp in plans:
                    tab = table[l][pp.q]
                    h, sub = pp.q // 2, pp.q % 2
                    rbase = 0 if sub == 0 else 128 * plans[2 * h].nblk
                    rodv = rod[h][rbase:rbase + P * pp.nblk, :] \
                        .rearrange("(l b) f -> l b f", l=P)
                    col_cum = 0
                    for (cols, groups) in pp.tiles:
                        mt = msgp.tile([P, cfg.tile_cols, dh], f32, tag="mt")
                        nidx = cols * P
                        slot0 = col_cum * P
                        nc.gpsimd.dma_gather(
                            mt[:, :cols, :],
                            tab[:, :],
                            gidx_s[pp.q][:, slot0 // 16:(slot0 + nidx) // 16],
                            nidx, nidx, dh, single_packet=False,
                            queue_num=next_q(),
                        )
                        blk_lo = groups[0][0]
                        blk_hi = groups[-1][0] + groups[-1][1]
                        rog = routp.tile([P, cfg.tile_cols, dh], f32,
                                         tag="rog")
                        for (blk0, nb, r, col0) in groups:
                            seg = mt[:, col0:col0 + nb * r, :]
                            seg = seg.rearrange("p (b r) f -> p b f r", r=r)
                            nc.vector.reduce_sum(
                                rog[:, blk0 - blk_lo:blk0 - blk_lo + nb, :],
                                seg, axis=mybir.AxisListType.X)
                        # write this tile's partial blocks (row = lane*nblk+b)
                        nc.sync.dma_start(
                            rodv[:, blk_lo:blk_hi, :],
                            rog[:, :blk_hi - blk_lo, :])
                        col_cum += cols

            def combine_wave(k, self_all, body):
                """Pre-gather wave k's per-pass partials, then run body(j, t)
                for each chunk j of the wave, with t = self + sum of partials
                for chunk j's rows."""
                lo = int(cfg.woff[k])
                wk = cfg.wsize[k]
                wc = wk // P
                wcm = max(w // P for w in cfg.wsize)
                parts = []
                for h in range(2):
                    pg = cmbp.tile([P, 2 * wcm, dh], f32, tag=f"pg{h}")
                    # split into two calls to stay within the desc ring
                    nc.gpsimd.dma_gather(
                        pg[:, :wc, :], rod[h][:, :],
                        pgidx_s[k][h][:, :wk // 16],
                        wk, wk, dh, single_packet=False,
                        queue_num=next_q(),
                    )
                    nc.gpsimd.dma_gather(
                        pg[:, wc:2 * wc, :], rod[h][:, :],
                        pgidx_s[k][h][:, wk // 16:2 * wk // 16],
                        wk, wk, dh, single_packet=False,
                        queue_num=next_q(),
                    )
                    red = cmbp.tile([P, wcm, dh], f32, tag=f"pr{h}")
                    nc.vector.reduce_sum(
                        red[:, :wc, :],
                        pg[:, :2 * wc, :].rearrange("p (a c) f -> p c f a",
                                                    a=2),
                        axis=mybir.AxisListType.X)
                    parts.append(red)
                part = cmbp.tile([P, wcm, dh], f32, tag="part")
                nc.vector.tensor_add(part[:, :wc, :], parts[0][:, :wc, :],
                                     parts[1][:, :wc, :])
                j0 = lo // P
                for cj in range(wc):
                    j = j0 + cj
                    if j >= nchunk:
                        break
                    nj = min(P, shard - j * P)
                    t = chp.tile([P, dh], f32, tag="t")
                    nc.vector.tensor_add(t[:nj, :], self_all[:nj, j, :],
                                         part[:nj, cj, :])
                    body(j, nj, t)

            do_passes(0)

            # ------- phase D: hid'' = leakyrelu(acc0*dinv + b1) * dinv
            def phase_d_body(j, nj, t):
                qj, off = chunk_quarter(j)
                t0 = chp.tile([P, dh], f32, tag="t0")
                nc.vector.tensor_scalar_mul(t0[:nj, :], t[:nj, :],
                                            dinv_s[:nj, j:j + 1])
                nc.vector.tensor_add(t0[:nj, :], t0[:nj, :], b1_s[:nj, :])
                t1 = chp.tile([P, dh], f32, tag="t1")
                nc.vector.tensor_scalar_mul(t1[:nj, :], t0[:nj, :], 0.01)
                nc.vector.tensor_max(t0[:nj, :], t0[:nj, :], t1[:nj, :])
                nc.vector.tensor_scalar_mul(hid_all[:nj, j, :], t0[:nj, :],
                                            dinv_s[:nj, j:j + 1])
                nc.sync.dma_start(ag_in[1][qj][off:off + nj, :],
                                  hid_all[:nj, j, :])
                if j == qend_chunk[qj]:
                    allgather(1, qj)

            for k in range(Q):
                combine_wave(k, hc_all, phase_d_body)

            do_passes(1)

            # ------- phase G: out = (acc1*dinv) @ W2 + b2
            def phase_g_body(j, nj, t):
                p2 = chp.tile([P, dh], f32, tag="p2")
                nc.vector.tensor_scalar_mul(p2[:nj, :], t[:nj, :],
                                            dinv_s[:nj, j:j + 1])
                ptr = psp.tile([dh, P], f32, tag="tr")
                nc.tensor.transpose(ptr[:, :nj], p2[:nj, :], ident[:nj, :nj])
                p2t = chp.tile([dh, P], f32, tag="p2t")
                nc.vector.tensor_copy(p2t[:, :nj], ptr[:, :nj])
                po = psp1.tile([P, dout], f32, tag="mo")
                nc.tensor.matmul(po[:nj, :], lhsT=p2t[:, :nj], rhs=w2_s[:],
                                 start=True, stop=True)
                oc = chp.tile([P, dout], f32, tag="oc")
                nc.vector.tensor_add(oc[:nj, :], po[:nj, :], b2_s[:nj, :])
                nc.sync.dma_start(outp[j * P:j * P + nj, :], oc[:nj, :])

            for k in range(Q):
                combine_wave(k, hid_all, phase_g_body)

    nc.compile()
    return nc


# ---------------------------------------------------------------- inputs per core

def make_in_maps(inputs, plans, cfg: Cfg):
    x = np.ascontiguousarray(np.asarray(inputs["x"], np.float32))
    W1 = np.ascontiguousarray(np.asarray(inputs["W1"], np.float32))
    b1 = np.ascontiguousarray(np.asarray(inputs["b1"], np.float32).reshape(1, -1))
    W2 = np.ascontiguousarray(np.asarray(inputs["W2"], np.float32))
    b2 = np.ascontiguousarray(np.asarray(inputs["b2"], np.float32).reshape(1, -1))
    _, deg = None, None  # deg recomputed below to avoid threading state
    dst = np.asarray(inputs["edge_index"][1], dtype=np.int64)
    degv = np.bincount(dst, minlength=cfg.N).astype(np.int64) + 1
    dinv = (1.0 / np.sqrt(degv)).astype(np.float32)

    in_maps = []
    for c in range(cfg.ncores):
        sl = slice(c * cfg.shard, (c + 1) * cfg.shard)
        xTc = np.ascontiguousarray(x[sl].T)
        dpad = np.zeros(cfg.shard_pad, np.float32)
        dpad[:cfg.shard] = dinv[sl]
        dcol = np.ascontiguousarray(dpad.reshape(cfg.nchunk, 128).T)
        m = {"xT": xTc, "W1": W1, "b1": b1, "W2": W2, "b2": b2,
             "dinv_col": dcol}
        for q in range(cfg.Q):
            m[f"gidx_p{q}"] = plans[q].gidx[c]
            for h in range(2):
                m[f"pgidx_k{q}h{h}"] = plans.pgidx[q][h][c]
        in_maps.append(m)
    return in_maps


# ---------------------------------------------------------------- entry point

def kernel(**inputs):
    from concourse.bass_utils import run_bass_kernel_spmd
    cfg = CFG
    plans, _deg = build_plan(inputs["edge_index"], cfg)
    nc = build_bass(plans, cfg)
    in_maps = make_in_maps(inputs, plans, cfg)
    core_ids = list(range(cfg.ncores))
    res = run_bass_kernel_spmd(nc, in_maps, core_ids).results
    out = np.concatenate([res[c]["out"] for c in core_ids], axis=0)
    return out.astype(np.float32)



# revision 34
# speedup vs baseline: 1.6742x; 1.3513x over previous
"""GCN 2-layer (PyG GCNConv x2) Trainium2 kernel, 8-core SPMD.

Strategy:
  - Shard destination nodes across the 8 cores (12500 each). Weights
    replicated. SWDGE descriptor generation is the machine bottleneck, so
    gathers/scatter-free combines round-robin over 4 SWDGE queues and the
    descriptor count is minimized.
  - Windows are source QUARTERS of every rank so the per-layer AllGather
    runs as 4 pipelined quarter-collectives overlapping phase compute and
    the first gather passes.
  - Layer l: each core computes its shard of the scaled features
    (h'' = (x @ W) * dinv for layer 1, hid'' = leakyrelu(...)*dinv for 2),
    kept resident in SBUF (self-loop term) and quarter-AllGathered into
    per-window tables in HBM.
  - Message gather: MoE dma_gather (int16 idxs) from the window table.
    Per (core, window) pass, destinations are sorted by in-window degree
    and laid out on the 128 SBUF partitions; each block of 128 destinations
    has a uniform round count R (max degree in the block); gathered
    messages land [128 lanes, R rounds, 64] and a strided reduce_sum
    collapses R into per-block partial sums.
  - Partial blocks are bulk-written (HWDGE, no descriptors on the SWDGE
    path) to per-pass-pair "rod" DRAM tensors. The combine step then
    gathers, per destination row, its 4 per-pass partials (one descriptor
    per row-pass instead of a 2-descriptor RMW scatter per row) and sums
    them with the SBUF-resident self term — fused into phase D (layer
    boundary) and phase G (output).
  - Layer 2 reuses the identical edge schedule/index tensors on the hid''
    table; final output = (acc2 * dinv) @ W2 + b2 via PE transpose+matmul.
"""

import numpy as np


# ---------------------------------------------------------------- config

class Cfg:
    """Windows are source-QUARTERS of every rank (quarter q of each shard),
    so AllGather can run as 4 pipelined quarter-collectives.

    Window q's table = concat over ranks of [qsize[q] rows + pad zeros].
    """
    def __init__(self, N=100000, E=1200000, ncores=8, Q=4, tile_cols=32,
                 din=128, dh=64, dout=40):
        assert N % ncores == 0
        self.N, self.E, self.ncores, self.Q = N, E, ncores, Q
        self.shard = N // ncores             # 12500
        self.pad_rows = 16                   # zero rows appended per stripe
        # 128-aligned quarter sizes summing to shard; quarter 0 is smaller so
        # the first AllGather (both layers) fires early and unblocks pass 0
        assert N == 100000 and ncores == 8 and Q == 4
        qs = [2560, 3328, 3328, 3284]
        self.qsize = qs
        assert sum(qs) == self.shard
        self.qoff = np.concatenate([[0], np.cumsum(qs)]).astype(np.int64)
        self.stripe_rows = [q + self.pad_rows for q in qs]
        self.window_rows = [self.ncores * sr for sr in self.stripe_rows]
        assert max(self.window_rows) <= 32767
        self.tile_cols = tile_cols           # msg tile free columns (rounds)
        self.din, self.dh, self.dout = din, dh, dout
        self.nchunk = (self.shard + 127) // 128
        self.shard_pad = self.nchunk * 128
        # combine waves: one per quarter, sizes padded up to 128 multiples
        self.wsize = [((q + 127) // 128) * 128 for q in self.qsize]
        self.woff = self.qoff[:Q]            # same starts as quarters

    def win_of(self, s):
        """Window (source quarter) of global source id array s."""
        return np.searchsorted(self.qoff, s % self.shard, side="right") - 1

    def src_local(self, s, q):
        """Window-local table row of global source id array s in window q."""
        rank = s // self.shard
        return rank * self.stripe_rows[q] + (s % self.shard - self.qoff[q])

    def zlocal(self, q):
        return self.qsize[q]                 # first zero row of rank 0 stripe


CFG = Cfg()


# ---------------------------------------------------------------- plan

def _wrap16(a):
    """Device idx layout: logical position i lives at [i % 16, i // 16];
    the 16-partition pattern is replicated across all 128 partitions
    (one copy per Q7 core)."""
    a = np.asarray(a, dtype=np.int16)
    assert a.size % 16 == 0
    w = np.ascontiguousarray(a.reshape(-1, 16).T)
    return np.ascontiguousarray(np.tile(w, (8, 1)))


class PassPlan:
    """Shared (cross-core) schedule + per-core index tensors for one
    (window) pass. The same schedule is reused by both layers."""
    __slots__ = ("q", "nblk", "R", "base", "S", "groups", "tiles",
                 "gidx")


class PlanSet(list):
    """List of PassPlan plus the combine-gather plan.

    Per-pass partial sums (ro blocks) are bulk-written to two DRAM "rod"
    tensors (passes 0+1 and 2+3 concatenated, plus a zero row block).
    The combine step gathers, per destination row, its per-pass partials:
    pgidx[k][h][c] holds, for wave k (quarter rows, padded to 128) and
    half h (passes 2h, 2h+1), the rod row of each (pass, dst) partial.
    rod row of rank j in pass p = pass_base + (j % 128) * nblk_p + j // 128.
    """
    __slots__ = ("pgidx", "rod_rows", "zrow")


def build_plan(edge_index, cfg: Cfg):
    """edge_index: [2, E] int array (sources row 0, destinations row 1).
    Returns (plan_list, deg) where plan_list has cfg.Q PassPlan entries."""
    src = np.asarray(edge_index[0], dtype=np.int64)
    dst = np.asarray(edge_index[1], dtype=np.int64)
    N, Q, ncores, shard = cfg.N, cfg.Q, cfg.ncores, cfg.shard

    deg = np.bincount(dst, minlength=N).astype(np.int64) + 1  # + self loop

    # Per (core, window) edge sets.
    core_of = dst // shard
    win_of = cfg.win_of(src)
    # order edges by (window, core) once
    order = np.lexsort((dst, core_of, win_of))
    src_s, dst_s = src[order], dst[order]
    wc_key = win_of[order] * ncores + core_of[order]
    seg_bounds = np.searchsorted(wc_key, np.arange(Q * ncores + 1))

    plans = []
    rank_pos = []   # rank_pos[q][c]: [shard] -> rank j in pass q's order, -1
    for q in range(Q):
        # per-core data for this window
        per_core = []
        for c in range(ncores):
            lo, hi = seg_bounds[q * ncores + c], seg_bounds[q * ncores + c + 1]
            s_loc = cfg.src_local(src_s[lo:hi], q)  # window-local table rows
            d = dst_s[lo:hi]                      # sorted by dst already
            uniq, counts = np.unique(d, return_counts=True)
            # sort destinations by count desc (stable for determinism)
            o = np.argsort(-counts, kind="stable")
            uniq, counts = uniq[o], counts[o]
            per_core.append((s_loc, d, uniq, counts))

        nblk = max((len(u) + 127) // 128 for (_, _, u, _) in per_core)
        nblk = max(nblk, 1)
        Rs = np.zeros(nblk, dtype=np.int64)
        for (_, _, uniq, counts) in per_core:
            nb = (len(uniq) + 127) // 128
            for j in range(nb):
                Rs[j] = max(Rs[j], counts[j * 128])
        Rs = np.maximum(Rs, 1)

        pp = PassPlan()
        pp.q = q
        pp.nblk = nblk
        pp.R = Rs
        pp.base = np.concatenate([[0], np.cumsum(Rs * 128)])
        pp.S = int(pp.base[-1])

        # merge equal-R runs into reduce groups, split into msg tiles
        tiles = []   # list of (cols, [(blk0, nb, R, col0_in_tile), ...])
        cur_groups, cur_cols = [], 0
        j = 0
        while j < nblk:
            r = Rs[j]
            nb_run = 1
            while j + nb_run < nblk and Rs[j + nb_run] == r:
                nb_run += 1
            # split run over tiles
            taken = 0
            while taken < nb_run:
                room = (cfg.tile_cols - cur_cols) // r
                if room <= 0:
                    tiles.append((cur_cols, cur_groups))
                    cur_groups, cur_cols = [], 0
                    room = cfg.tile_cols // r
                    assert room > 0, f"R={r} exceeds tile_cols={cfg.tile_cols}"
                nb_t = min(room, nb_run - taken)
                cur_groups.append((j + taken, nb_t, int(r), cur_cols))
                cur_cols += nb_t * int(r)
                taken += nb_t
            j += nb_run
        if cur_groups:
            tiles.append((cur_cols, cur_groups))
        pp.groups = None
        pp.tiles = tiles

        # per-core index tensors
        pp.gidx = []
        rank_pos.append([])
        for c in range(ncores):
            s_loc, d, uniq, counts = per_core[c]
            gi = np.full(pp.S, cfg.zlocal(q), dtype=np.int64)  # zero row default
            rp = np.full(shard, -1, dtype=np.int64)
            if len(uniq):
                rp[uniq - c * shard] = np.arange(len(uniq))
                rnk = rp[d - c * shard]
                # r index within each destination: edges sorted by dst; order
                # them by rank (stable) so positions within a rank are 0..cnt-1
                o2 = np.argsort(rnk, kind="stable")
                rnk_o = rnk[o2]
                s_o = s_loc[o2]
                starts = np.searchsorted(rnk_o, np.arange(len(uniq)))
                rwithin = np.arange(len(rnk_o)) - starts[rnk_o]
                blk = rnk_o // 128
                lane = rnk_o % 128
                slot = pp.base[blk] + rwithin * 128 + lane
                gi[slot] = s_o
            pp.gidx.append(_wrap16(gi))
            rank_pos[q].append(rp)
        plans.append(pp)

    # ---- combine-gather plan (replaces scatter_add)
    ps = PlanSet(plans)
    ps.rod_rows = []
    ps.zrow = []
    for h in range(2):
        nba, nbb = plans[2 * h].nblk, plans[2 * h + 1].nblk
        ps.zrow.append(128 * (nba + nbb))
        ps.rod_rows.append(128 * (nba + nbb) + 16)
        assert ps.zrow[h] <= 32767
    ps.pgidx = []
    for k in range(Q):
        lo = int(cfg.woff[k])
        wk = cfg.wsize[k]
        halves = []
        for h in range(2):
            pa, pb = 2 * h, 2 * h + 1
            nba, nbb = plans[pa].nblk, plans[pb].nblk
            Z = ps.zrow[h]
            per_core_idx = []
            for c in range(ncores):
                idx = np.full(2 * wk, Z, dtype=np.int64)
                rows = np.arange(lo, min(lo + wk, shard))
                for pl, (p, nb, base) in enumerate(
                        [(pa, nba, 0), (pb, nbb, 128 * nba)]):
                    j = rank_pos[p][c][rows]
                    v = np.where(j >= 0, base + (j % 128) * nb + j // 128, Z)
                    idx[pl * wk:pl * wk + len(rows)] = v
                per_core_idx.append(_wrap16(idx))
            halves.append(per_core_idx)
        ps.pgidx.append(halves)
    return ps, deg


# ---------------------------------------------------------------- numpy golden
# (mirrors device semantics exactly; used for development/testing)

def golden(inputs, cfg: Cfg = CFG):
    x = np.asarray(inputs["x"], np.float32)
    ei = np.asarray(inputs["edge_index"])
    W1 = np.asarray(inputs["W1"], np.float32)
    b1 = np.asarray(inputs["b1"], np.float32)
    W2 = np.asarray(inputs["W2"], np.float32)
    b2 = np.asarray(inputs["b2"], np.float32)
    plans, deg = build_plan(ei, cfg)
    dinv = (1.0 / np.sqrt(deg)).astype(np.float32)

    def windowed(tab, q):
        # [N, d] -> [window_rows[q], d] for window q (source quarter q of
        # every rank, each stripe padded with zero rows)
        d = tab.shape[1]
        t = np.zeros((cfg.ncores, cfg.stripe_rows[q], d), np.float32)
        lo, hi = cfg.qoff[q], cfg.qoff[q + 1]
        t[:, :cfg.qsize[q]] = tab.reshape(cfg.ncores, cfg.shard, d)[:, lo:hi]
        return t.reshape(cfg.window_rows[q], d)

    def propagate(table_full):
        """table_full: [N, d] scaled source features. Returns [N, d] sums of
        incoming messages + self term. Mirrors the device rod/combine path."""
        d_feat = table_full.shape[1]
        out = np.zeros((cfg.N, d_feat), np.float32)
        for c in range(cfg.ncores):
            # per-pass partial blocks -> rod arrays
            rods = [np.zeros((plans.rod_rows[h], d_feat), np.float32)
                    for h in range(2)]
            for pp in plans:
                tabw = windowed(table_full, pp.q)
                gi = pp.gidx[c][:16].T.reshape(-1)      # unwrap
                msg = tabw[gi]                     # [S, d]
                h, sub = pp.q // 2, pp.q % 2
                base = 0 if sub == 0 else 128 * plans[2 * h].nblk
                nb = pp.nblk
                lanes = np.arange(128) * nb
                for j in range(pp.nblk):
                    r = int(pp.R[j])
                    seg = msg[pp.base[j]:pp.base[j + 1]].reshape(
                        r, 128, d_feat).sum(0)       # [128 lanes, d]
                    rods[h][base + lanes + j] = seg
            # combine: self + gathered per-pass partials
            a = table_full[c * cfg.shard:(c + 1) * cfg.shard].copy()
            for k in range(cfg.Q):
                lo = int(cfg.woff[k])
                wk = cfg.wsize[k]
                nrows = min(lo + wk, cfg.shard) - lo
                for h in range(2):
                    idx = plans.pgidx[k][h][c][:16].T.reshape(-1)
                    vals = rods[h][idx]              # [2*wk, d]
                    part = vals[:wk] + vals[wk:]
                    a[lo:lo + nrows] += part[:nrows]
            out[c * cfg.shard:(c + 1) * cfg.shard] = a
        return out

    hpp = (x @ W1) * dinv[:, None]
    acc1 = propagate(hpp)
    hid = acc1 * dinv[:, None] + b1
    hid = np.where(hid > 0, hid, 0.01 * hid)
    hpp2 = hid * dinv[:, None]
    acc2 = propagate(hpp2)
    return (acc2 * dinv[:, None]) @ W2 + b2


# ---------------------------------------------------------------- bass program

def build_bass(plans, cfg: Cfg, debug=False):
    import concourse.bass as bass
    import concourse.mybir as mybir
    import concourse.tile as tile
    from concourse import bacc
    from concourse.masks import make_identity

    f32 = mybir.dt.float32
    i16 = mybir.dt.int16
    P = 128
    shard, Q, nchunk = cfg.shard, cfg.Q, cfg.nchunk
    dh, dout = cfg.dh, cfg.dout

    nc = bacc.Bacc(None, target_bir_lowering=False, debug=debug,
                   num_swdge_queues=4, dynamic_dma_scratch_size=32768)

    # ---- external I/O (per-core shapes; SPMD-uniform)
    xT = nc.declare_dram_parameter("xT", [cfg.din, shard], f32, isOutput=False)
    W1p = nc.declare_dram_parameter("W1", [cfg.din, dh], f32, isOutput=False)
    b1p = nc.declare_dram_parameter("b1", [1, dh], f32, isOutput=False)
    W2p = nc.declare_dram_parameter("W2", [dh, dout], f32, isOutput=False)
    b2p = nc.declare_dram_parameter("b2", [1, dout], f32, isOutput=False)
    dinvp = nc.declare_dram_parameter("dinv_col", [P, nchunk], f32, isOutput=False)
    gidxp = [nc.declare_dram_parameter(f"gidx_p{q}", list(plans[q].gidx[0].shape),
                                       i16, isOutput=False) for q in range(Q)]
    pgidxp = [[nc.declare_dram_parameter(
        f"pgidx_k{k}h{h}", list(plans.pgidx[k][h][0].shape), i16,
        isOutput=False) for h in range(2)] for k in range(Q)]
    outp = nc.declare_dram_parameter("out", [shard, dout], f32, isOutput=True)

    # ---- internal DRAM (per-layer, per-quarter-window)
    ag_in = [[nc.dram_tensor(f"ag_in{l}_{q}", [cfg.stripe_rows[q], dh], f32)
              for q in range(Q)] for l in (0, 1)]
    table = [[nc.dram_tensor(f"table{l}_{q}",
                             [cfg.ncores * cfg.stripe_rows[q], dh], f32,
                             addr_space="Shared") for q in range(Q)]
             for l in (0, 1)]
    # per-pass partial blocks (passes 0+1 / 2+3 concatenated + zero rows);
    # reused by both layers
    rod = [nc.dram_tensor(f"rod{h}", [plans.rod_rows[h], dh], f32)
           for h in range(2)]

    core_ids = list(range(cfg.ncores))

    # chunk j (rows 128j..) -> (quarter q, row offset within quarter);
    # quarter boundaries are 128-aligned except the final end.
    def chunk_quarter(j):
        row0 = j * 128
        q = int(np.searchsorted(cfg.qoff, row0, side="right") - 1)
        return q, row0 - int(cfg.qoff[q])

    qend_chunk = [int((cfg.qoff[q + 1] - 1) // 128) for q in range(Q)]

    with tile.TileContext(nc) as tc:
        with (
            tc.tile_pool(name="const", bufs=1) as constp,
            tc.tile_pool(name="big", bufs=1) as bigp,
            tc.tile_pool(name="chunk", bufs=3) as chp,
            tc.tile_pool(name="msg", bufs=4) as msgp,
            tc.tile_pool(name="rout", bufs=2) as routp,
            tc.tile_pool(name="cmb", bufs=1) as cmbp,
            tc.tile_pool(name="psum", bufs=2, space="PSUM") as psp,
            tc.tile_pool(name="psum1", bufs=2, space="PSUM") as psp1,
        ):
            # ------- constants
            w1_s = constp.tile([cfg.din, dh], f32)
            nc.sync.dma_start(w1_s[:], W1p[:])
            w2_s = constp.tile([dh, dout], f32)
            nc.sync.dma_start(w2_s[:], W2p[:])
            b1_s = constp.tile([P, dh], f32)
            nc.sync.dma_start(b1_s[:], b1p[:1, :].to_broadcast((P, dh)))
            b2_s = constp.tile([P, dout], f32)
            nc.sync.dma_start(b2_s[:], b2p[:1, :].to_broadcast((P, dout)))
            dinv_s = constp.tile([P, nchunk], f32)
            nc.sync.dma_start(dinv_s[:], dinvp[:])
            ident = constp.tile([P, P], f32)
            make_identity(nc, ident[:])
            zpad = constp.tile([cfg.pad_rows, dh], f32)
            nc.vector.memset(zpad[:], 0.0)
            for l in (0, 1):
                for q in range(Q):
                    nc.sync.dma_start(
                        ag_in[l][q][cfg.qsize[q]:cfg.stripe_rows[q], :],
                        zpad[:])

            # gather/combine index tiles (resident; reused by both layers)
            gidx_s = []
            pgidx_s = []
            for q in range(Q):
                g = constp.tile(list(plans[q].gidx[0].shape), i16, tag=f"gidx{q}")
                nc.sync.dma_start(g[:], gidxp[q][:])
                gidx_s.append(g)
                hs = []
                for h in range(2):
                    s = constp.tile(list(plans.pgidx[q][h][0].shape), i16,
                                    tag=f"pgidx{q}_{h}")
                    nc.sync.dma_start(s[:], pgidxp[q][h][:])
                    hs.append(s)
                pgidx_s.append(hs)
            # zero the rod zero-row blocks (tail 16 rows; layer-shared)
            for h in range(2):
                nc.sync.dma_start(rod[h][plans.zrow[h]:plans.rod_rows[h], :],
                                  zpad[:])

            def allgather(l, q):
                nc.gpsimd.collective_compute(
                    "AllGather", mybir.AluOpType.bypass,
                    replica_groups=[core_ids],
                    ins=[ag_in[l][q][:].opt()],
                    outs=[table[l][q][:].opt()],
                )

            # ------- phase A: h'' = (x @ W1) * dinv, write ag_in0 + acc0 prefill
            # self terms stay resident in SBUF (h'' for layer 1, hid'' for 2)
            hc_all = bigp.tile([P, nchunk, dh], f32, tag="hc_all")
            hid_all = bigp.tile([P, nchunk, dh], f32, tag="hid_all")

            for j in range(nchunk):
                nj = min(P, shard - j * P)
                qj, off = chunk_quarter(j)
                xc = chp.tile([cfg.din, P], f32, tag="xc")
                nc.sync.dma_start(xc[:, :nj], xT[:, j * P:j * P + nj])
                ph = psp.tile([P, dh], f32, tag="mm")
                nc.tensor.matmul(ph[:nj, :], lhsT=xc[:, :nj],
                                 rhs=w1_s[:], start=True, stop=True)
                nc.vector.tensor_scalar_mul(hc_all[:nj, j, :], ph[:nj, :],
                                            dinv_s[:nj, j:j + 1])
                nc.sync.dma_start(ag_in[0][qj][off:off + nj, :],
                                  hc_all[:nj, j, :])
                if j == qend_chunk[qj]:
                    allgather(0, qj)

            qctr = [0]

            def next_q():
                q = qctr[0] % 4
                qctr[0] += 1
                return q

            def do_passes(l):
                for pp in plans:
                    tab = table[l][pp.q]
                    h, sub = pp.q // 2, pp.q % 2
                    rbase = 0 if sub == 0 else 128 * plans[2 * h].nblk
                    rodv = rod[h][rbase:rbase + P * pp.nblk, :] \
                        .rearrange("(l b) f -> l b f", l=P)
                    col_cum = 0
                    for (cols, groups) in pp.tiles:
                        mt = msgp.tile([P, cfg.tile_cols, dh], f32, tag="mt")
                        nidx = cols * P
                        slot0 = col_cum * P
                        nc.gpsimd.dma_gather(
                            mt[:, :cols, :],
                            tab[:, :],
                            gidx_s[pp.q][:, slot0 // 16:(slot0 + nidx) // 16],
                            nidx, nidx, dh, single_packet=False,
                            queue_num=next_q(),
                        )
                        blk_lo = groups[0][0]
                        blk_hi = groups[-1][0] + groups[-1][1]
                        rog = routp.tile([P, cfg.tile_cols, dh], f32,
                                         tag="rog")
                        for (blk0, nb, r, col0) in groups:
                            seg = mt[:, col0:col0 + nb * r, :]
                            seg = seg.rearrange("p (b r) f -> p b f r", r=r)
                            nc.vector.reduce_sum(
                                rog[:, blk0 - blk_lo:blk0 - blk_lo + nb, :],
                                seg, axis=mybir.AxisListType.X)
                        # write this tile's partial blocks (row = lane*nblk+b)
                        nc.sync.dma_start(
                            rodv[:, blk_lo:blk_hi, :],
                            rog[:, :blk_hi - blk_lo, :])
                        col_cum += cols

            def combine_wave(k, self_all, body):
                """Pre-gather wave k's per-pass partials, then run body(j, t)
                for each chunk j of the wave, with t = self + sum of partials
                for chunk j's rows."""
                lo = int(cfg.woff[k])
                wk = cfg.wsize[k]
                wc = wk // P
                wcm = max(w // P for w in cfg.wsize)
                parts = []
                for h in range(2):
                    pg = cmbp.tile([P, 2 * wcm, dh], f32, tag=f"pg{h}")
                    # split into two calls to stay within the desc ring
                    nc.gpsimd.dma_gather(
                        pg[:, :wc, :], rod[h][:, :],
                        pgidx_s[k][h][:, :wk // 16],
                        wk, wk, dh, single_packet=False,
                        queue_num=next_q(),
                    )
                    nc.gpsimd.dma_gather(
                        pg[:, wc:2 * wc, :], rod[h][:, :],
                        pgidx_s[k][h][:, wk // 16:2 * wk // 16],
                        wk, wk, dh, single_packet=False,
                        queue_num=next_q(),
                    )
                    red = cmbp.tile([P, wcm, dh], f32, tag=f"pr{h}")
                    nc.vector.reduce_sum(
                        red[:, :wc, :],
                        pg[:, :2 * wc, :].rearrange("p (a c) f -> p c f a",
                                                    a=2),
                        axis=mybir.AxisListType.X)
                    parts.append(red)
                part = cmbp.tile([P, wcm, dh], f32, tag="part")
                nc.vector.tensor_add(part[:, :wc, :], parts[0][:, :wc, :],
                                     parts[1][:, :wc, :])
                j0 = lo // P
                for cj in range(wc):
                    j = j0 + cj
                    if j >= nchunk:
                        break
                    nj = min(P, shard - j * P)
                    t = chp.tile([P, dh], f32, tag="t")
                    nc.vector.tensor_add(t[:nj, :], self_all[:nj, j, :],
                                         part[:nj, cj, :])
                    body(j, nj, t)

            do_passes(0)

            # ------- phase D: hid'' = leakyrelu(acc0*dinv + b1) * dinv
            def phase_d_body(j, nj, t):
                qj, off = chunk_quarter(j)
                t0 = chp.tile([P, dh], f32, tag="t0")
                nc.vector.tensor_scalar_mul(t0[:nj, :], t[:nj, :],
                                            dinv_s[:nj, j:j + 1])
                nc.vector.tensor_add(t0[:nj, :], t0[:nj, :], b1_s[:nj, :])
                t1 = chp.tile([P, dh], f32, tag="t1")
                nc.vector.tensor_scalar_mul(t1[:nj, :], t0[:nj, :], 0.01)
                nc.vector.tensor_max(t0[:nj, :], t0[:nj, :], t1[:nj, :])
                nc.vector.tensor_scalar_mul(hid_all[:nj, j, :], t0[:nj, :],
                                            dinv_s[:nj, j:j + 1])
                nc.sync.dma_start(ag_in[1][qj][off:off + nj, :],
                                  hid_all[:nj, j, :])
                if j == qend_chunk[qj]:
                    allgather(1, qj)

            for k in range(Q):
                combine_wave(k, hc_all, phase_d_body)

            do_passes(1)

            # ------- phase G: out = (acc1*dinv) @ W2 + b2
            def phase_g_body(j, nj, t):
                p2 = chp.tile([P, dh], f32, tag="p2")
                nc.vector.tensor_scalar_mul(p2[:nj, :], t[:nj, :],
                                            dinv_s[:nj, j:j + 1])
                ptr = psp.tile([dh, P], f32, tag="tr")
                nc.tensor.transpose(ptr[:, :nj], p2[:nj, :], ident[:nj, :nj])
                p2t = chp.tile([dh, P], f32, tag="p2t")
                nc.vector.tensor_copy(p2t[:, :nj], ptr[:, :nj])
                po = psp1.tile([P, dout], f32, tag="mo")
                nc.tensor.matmul(po[:nj, :], lhsT=p2t[:, :nj], rhs=w2_s[:],
                                 start=True, stop=True)
                oc = chp.tile([P, dout], f32, tag="oc")
                nc.vector.tensor_add(oc[:nj, :], po[:nj, :], b2_s[:nj, :])
                nc.sync.dma_start(outp[j * P:j * P + nj, :], oc[:nj, :])

            for k in range(Q):
                combine_wave(k, hid_all, phase_g_body)

    nc.compile()
    return nc


# ---------------------------------------------------------------- inputs per core

def make_in_maps(inputs, plans, cfg: Cfg):
    x = np.ascontiguousarray(np.asarray(inputs["x"], np.float32))
    W1 = np.ascontiguousarray(np.asarray(inputs["W1"], np.float32))
    b1 = np.ascontiguousarray(np.asarray(inputs["b1"], np.float32).reshape(1, -1))
    W2 = np.ascontiguousarray(np.asarray(inputs["W2"], np.float32))
    b2 = np.ascontiguousarray(np.asarray(inputs["b2"], np.float32).reshape(1, -1))
    _, deg = None, None  # deg recomputed below to avoid threading state
    dst = np.asarray(inputs["edge_index"][1], dtype=np.int64)
    degv = np.bincount(dst, minlength=cfg.N).astype(np.int64) + 1
    dinv = (1.0 / np.sqrt(degv)).astype(np.float32)

    in_maps = []
    for c in range(cfg.ncores):
        sl = slice(c * cfg.shard, (c + 1) * cfg.shard)
        xTc = np.ascontiguousarray(x[sl].T)
        dpad = np.zeros(cfg.shard_pad, np.float32)
        dpad[:cfg.shard] = dinv[sl]
        dcol = np.ascontiguousarray(dpad.reshape(cfg.nchunk, 128).T)
        m = {"xT": xTc, "W1": W1, "b1": b1, "W2": W2, "b2": b2,
             "dinv_col": dcol}
        for q in range(cfg.Q):
            m[f"gidx_p{q}"] = plans[q].gidx[c]
            for h in range(2):
                m[f"pgidx_k{q}h{h}"] = plans.pgidx[q][h][c]
        in_maps.append(m)
    return in_maps


# ---------------------------------------------------------------- entry point

def kernel(**inputs):
    from concourse.bass_utils import run_bass_kernel_spmd
    cfg = CFG
    plans, _deg = build_plan(inputs["edge_index"], cfg)
    nc = build_bass(plans, cfg)
    in_maps = make_in_maps(inputs, plans, cfg)
    core_ids = list(range(cfg.ncores))
    res = run_bass_kernel_spmd(nc, in_maps, core_ids).results
    out = np.concatenate([res[c]["out"] for c in core_ids], axis=0)
    return out.astype(np.float32)

